# revision 18
# baseline (speedup 1.0000x reference)
"""Block-sparse matmul + bias + relu on 8 Trainium2 NeuronCores.

Strategy (data-parallel over batch):
  - Shard x along batch: 8 cores x 512 rows. w_blocks/bias replicated.
  - Per core, compute out^T: x^T resident in SBUF as [128, 32, 512] (input
    block i at partition strip 32*(i%4), free tile i//4); each nonzero block
    (i,j) is one 32x32-tile matmul (lhsT = w block, rhs = x^T block strip,
    N=512 batch).  The PE is per-instruction-bound (~33ns/block), so the
    schedule exists to keep all 16 tiles streaming while minimizing sync.
  - Single-bank accumulation: all 16 (strip, col) cells of an output quad
    accumulate into ONE PSUM bank by separating row groups in time.  Groups
    of 8 quads (8 banks) run 4 rounds with a phase rotation (quad a does
    row group (k+a)%4 in round k), so the 16 PE tiles stay concurrent
    across quads while each quad's bank sees only one row group at a time.
  - Hazard: two row-group tiles must never stream into one PSUM bank
    simultaneously (hardware errors out).  Each quad's round-k first MM
    therefore carries a semaphore wait on the completion of that quad's
    round-(k-1) last MM (_inject_round_waits); groups of 8 make the natural
    separation large enough that these waits almost never stall.
  - Epilogue per quad: one DVE tensor_scalar relu(acc + bias) -> bf16, then
    DMA out^T.  (ScalarE is ~2.3x off spec on TRN2; DVE single-src runs 2x.)
  - Post-passes: offload multi-waits onto NoOps; thin per-MM semaphore
    increments to sem-add-imm jumps at just the waited-on counts.
  - Host: transpose/cast prep (bf16 feeds the PE; fp32 accumulate in PSUM).
"""

import os

import numpy as np
import ml_dtypes

import concourse.bass as bass
import concourse.tile as tile
from concourse import mybir
from concourse.bass_utils import run_bass_kernel_spmd

LAST_RESULTS = None  # test-only: BassKernelResults of the last run

BS = 32
KB = 128
NB = 128
BATCH = 4096
NCORES = 8
BC = BATCH // NCORES          # 512 batch rows per core
NQ = NB // 4                  # 32 quads of output block-cols
GQ = 8                        # quads per group (= PSUM banks used)
NG = NQ // GQ                 # 4 groups of 8 quads
if os.environ.get("BASS_KERNEL_F32R"):
    IN_DT = mybir.dt.float32r
    IN_NP = np.float32
else:
    IN_DT = mybir.dt.bfloat16
    IN_NP = ml_dtypes.bfloat16
if os.environ.get("BASS_KERNEL_OUT_F32"):
    OUT_DT = mybir.dt.float32
    OUT_NP = np.float32
else:
    OUT_DT = mybir.dt.bfloat16
    OUT_NP = ml_dtypes.bfloat16

_CACHE = {}


def _build_schedule(row_idx, col_idx):
    """Latin-square phased schedule.

    Returns (sched, S, slot_of, dummy_slots):
      sched[g] = list of (q, p, c, t, slot, start, stop) in emission order
        for quad group g (quads 4g..4g+3); p is the row group (i%4) and the
        PE tile is (32p, 32c); all 16 cells of quad q accumulate into ONE
        PSUM bank.
      S = per-strip slot count of the weight image.
    """
    nnz = len(row_idx)
    cells = [[[[] for _ in range(4)] for _ in range(4)] for _ in range(NQ)]
    for n in range(nnz):
        i = int(row_idx[n]); j = int(col_idx[n])
        cells[j // 4][i % 4][j % 4].append(n)

    slot_ctr = [0, 0, 0, 0]
    slot_of = {}
    dummy_slots = []
    sched = []
    bounds = []   # (wait_on_gidx, at_gidx): at-MM must wait completion of
                  # wait_on-MM (same quad's previous round last MM) so two
                  # row-group tiles never stream into one PSUM bank at once
    gidx = 0
    for g in range(NG):
        quads = [GQ * g + a for a in range(GQ)]
        # every (q, c) region needs >= 1 MM so the PSUM strip is defined
        for q in quads:
            for c in range(4):
                if not any(cells[q][p][c] for p in range(4)):
                    cells[q][0][c].append(None)
        ent = []
        last_of_quad = {}   # quad -> gidx of its last MM emitted so far
        for k in range(4):
            for a, q in enumerate(quads):
                p = (k + a) % 4
                maxd = max(len(cells[q][p][c]) for c in range(4))
                firstq = True
                for s in range(maxd):
                    for c in range(4):
                        lst = cells[q][p][c]
                        if s < len(lst):
                            n = lst[s]
                            slot = slot_ctr[p]
                            slot_ctr[p] += 1
                            if n is None:
                                dummy_slots.append((p, slot))
                                t = 0
                            else:
                                slot_of[n] = slot
                                t = int(row_idx[n]) // 4
                            if firstq and q in last_of_quad:
                                bounds.append((last_of_quad[q], gidx))
                            firstq = False
                            last_of_quad[q] = gidx
                            ent.append([q, p, c, t, slot, False, False])
                            gidx += 1
        # start/stop per (q, c) region in emission order
        first = {}
        last = {}
        for idx, e in enumerate(ent):
            key = (e[0], e[2])
            if key not in first:
                first[key] = idx
            last[key] = idx
        for idx in first.values():
            ent[idx][5] = True
        for idx in last.values():
            ent[idx][6] = True
        sched.append([tuple(e) for e in ent])
    S = max(slot_ctr)
    return sched, S, slot_of, dummy_slots, bounds


_MULTIWAIT_OK = {"InstDMACopy", "InstUnconditionalBranch",
                 "InstConditionalBranch"}


def _legalize_waits(nc):
    """Engine ISA structs carry a single sync-wait slot; Tile can emit more.
    Offload excess waits onto same-engine NoOps inserted just before the
    instruction (per-engine stream order is the block list order)."""
    ctr = 0
    for f in nc.m.functions:
        for blk in f.blocks:
            out = []
            for inst in blk.instructions:
                si = inst.sync_info
                if (si is not None and si.on_wait and len(si.on_wait) > 1
                        and type(inst).__name__ == "InstDMACopy"):
                    # HWDGE lane sems are monotonic add-only counters; a
                    # DMA's wait on its own completion lane orders it against
                    # unrelated prior DMAs on that lane and is droppable.
                    own = {u.ant_name for u in (si.on_update or [])}
                    keep = [w for w in si.on_wait if w.ant_name not in own]
                    if len(keep) > 1:
                        raise RuntimeError(
                            f"DMA {inst.name} still has waits {keep}")
                    inst.sync_info = mybir.SyncInfo(on_wait=keep,
                                                    on_update=si.on_update)
                    out.append(inst)
                    continue
                if (si is not None and si.on_wait and len(si.on_wait) > 1
                        and type(inst).__name__ not in _MULTIWAIT_OK):
                    waits = list(si.on_wait)
                    for w in waits[:-1]:
                        nop = mybir.InstNoOp(name=f"waitnop-{ctr}")
                        ctr += 1
                        nop.engine = inst.engine
                        nop.sync_info = mybir.SyncInfo(on_wait=[w], on_update=[])
                        out.append(nop)
                    inst.sync_info = mybir.SyncInfo(on_wait=[waits[-1]],
                                                    on_update=si.on_update)
                out.append(inst)
            blk.instructions[:] = out


def _inject_round_waits(nc, bounds, n_mm_per_rep):
    """Make each quad's round-k first MM wait for the completion (semaphore
    count) of that quad's round-(k-1) last MM.  An MM's PE-lane increment
    fires after its PSUM drain, so this guarantees two row-group tiles never
    stream into the same PSUM bank simultaneously.  Must run before
    _legalize_waits (multi-wait fixup) and _thin_pe_incs (which preserves all
    waited-on counts)."""
    for f in nc.m.functions:
        for blk in f.blocks:
            mms = [i for i in blk.instructions
                   if type(i).__name__ == "InstMatmult"
                   and getattr(i, "engine", None) == mybir.EngineType.PE]
            if len(mms) < n_mm_per_rep:
                continue
            assert len(mms) % n_mm_per_rep == 0, (len(mms), n_mm_per_rep)
            # PE lane sem template from any MM's update
            tmpl = None
            for i in mms:
                si = i.sync_info
                for u in (si.on_update or []) if si else []:
                    if u.ant_name.startswith("PE_"):
                        tmpl = u
                        break
                if tmpl:
                    break
            assert tmpl is not None
            nrep = len(mms) // n_mm_per_rep
            for rep in range(nrep):
                off = rep * n_mm_per_rep
                for (wait_on, at) in bounds:
                    inst = mms[off + at]
                    si = inst.sync_info or mybir.SyncInfo(on_wait=[],
                                                          on_update=[])
                    w = mybir.SyncWait(sync_type="semaphore", id=tmpl.id,
                                       ant_name=tmpl.ant_name,
                                       wait_mode="sem-ge-imm",
                                       wait_value=off + wait_on + 1)
                    inst.sync_info = mybir.SyncInfo(
                        on_wait=list(si.on_wait or []) + [w],
                        on_update=si.on_update)


def _thin_pe_incs(nc, lane_prefix="PE_"):
    """Per-MM semaphore increments serialize on the PE EVT_SEM port.  Since
    the PE completes instructions in pc order, the counting semaphore only
    needs to move at values someone actually waits on: keep an increment
    exactly at each waited cumulative count (as a sem-add-imm jump covering
    the dropped increments before it) plus the final one.  Every wait keeps
    its original literal value; each waited value is reached when the SAME
    instruction (or a later one) completes, so ordering is preserved."""
    waited = {}
    for f in nc.m.functions:
        for blk in f.blocks:
            for inst in blk.instructions:
                si = inst.sync_info
                if not si:
                    continue
                for w in (si.on_wait or []):
                    if w.ant_name.startswith(lane_prefix):
                        assert w.wait_mode == "sem-ge-imm", w
                        waited.setdefault(w.ant_name, set()).add(w.wait_value)
    for f in nc.m.functions:
        for blk in f.blocks:
            incs = {}
            for inst in blk.instructions:
                si = inst.sync_info
                if not si or not si.on_update:
                    continue
                if type(inst).__name__ == "InstEventSemaphore":
                    continue  # protocol add/sub bookkeeping: leave alone
                for u in si.on_update:
                    if (u.ant_name.startswith(lane_prefix)
                            and u.update_mode == "sem-inc"):
                        incs.setdefault(u.ant_name, []).append((inst, u))
            for sem, lst in incs.items():
                vset = waited.get(sem, set())
                cum = 0
                pending = 0
                for k, (inst, u) in enumerate(lst):
                    cum += 1
                    pending += 1
                    if cum in vset or k == len(lst) - 1:
                        u.update_mode = "sem-add-imm"
                        u.update_value = pending
                        pending = 0
                    else:
                        si = inst.sync_info
                        keep = [x for x in si.on_update if x is not u]
                        inst.sync_info = mybir.SyncInfo(on_wait=si.on_wait,
                                                        on_update=keep)
                assert pending == 0


def _build_program(sched, S, repeat=1, loop_n=0, bounds=None):
    nc = bass.Bass("TRN2", target_bir_lowering=False, debug=False,
                   num_devices=NCORES)
    x_d = nc.dram_tensor("xt", [128, 32 * BC], IN_DT, kind="ExternalInput").ap()
    w_d = nc.dram_tensor("wim", [128, S * 32], IN_DT, kind="ExternalInput").ap()
    b_d = nc.dram_tensor("bias", [128, 32], mybir.dt.float32,
                         kind="ExternalInput").ap()
    o_d = nc.dram_tensor("outT", [NQ, 128, BC], OUT_DT, kind="ExternalOutput").ap()

    import contextlib

    with tile.TileContext(nc) as tc:
        loop_cm = tc.For_i(0, loop_n, 1) if loop_n else contextlib.nullcontext()
        with tc.tile_pool(name="const", bufs=2) as cpool, \
             tc.tile_pool(name="work", bufs=4) as wpool, \
             tc.tile_pool(name="psum", bufs=1, space="PSUM") as ppool, \
             loop_cm:
            for rep in range(repeat):
              # per-sweep input tiles: with bufs=2 consecutive sweeps rotate
              # buffers, so sweep r+1's input DMA overlaps sweep r's compute
              xt = cpool.tile([128, 32 * BC], IN_DT, tag="xt",
                              name=f"xt_p{rep}")
              wt = cpool.tile([128, S * 32], IN_DT, tag="wt",
                              name=f"wt_p{rep}")
              bt = cpool.tile([128, 32], mybir.dt.float32, tag="bt",
                              name=f"bt_p{rep}")
              nc.sync.dma_start(bt[:], b_d[:])
              # x: chunked DMA (8 x 2MB)
              xch = (32 * BC) // 8
              for k in range(8):
                nc.sync.dma_start(xt[:, k * xch:(k + 1) * xch],
                                  x_d[:, k * xch:(k + 1) * xch])
              # w: chunked DMA in slot order so early groups unblock early
              wch = 4
              wstep = -(-S // wch) * 32
              for k in range(wch):
                lo = k * wstep
                hi = min(S * 32, lo + wstep)
                if lo >= hi:
                    continue
                nc.sync.dma_start(wt[:, lo:hi], w_d[:, lo:hi])

              for g in range(NG):
                acc = [ppool.tile([128, BC], mybir.dt.float32, tag=f"acc{a}",
                                  name=f"acc{a}_g{g}_p{rep}")
                       for a in range(GQ)]
                for (q, p, c, t, slot, start, stop) in sched[g]:
                    nc.tensor.matmul(
                        out=acc[q % GQ][32 * c:32 * c + 32, :],
                        lhsT=wt[32 * p:32 * p + 32,
                                slot * 32:(slot + 1) * 32],
                        rhs=xt[32 * p:32 * p + 32, t * BC:(t + 1) * BC],
                        start=start, stop=stop,
                        tile_position=(32 * p, 32 * c),
                        skip_group_check=True,
                    )
                for a in range(GQ):
                    q = GQ * g + a
                    ot = wpool.tile([128, BC], OUT_DT, tag=f"ot{a}",
                                    name=f"ot{a}_g{g}_p{rep}")
                    # bias+relu: (acc + bias_col) max 0.0 — DVE tensor_scalar
                    # (single-src 2x path); ScalarE is ~2.3x off spec (TRN2
                    # errata) so keep it off the critical path
                    nc.vector.tensor_scalar(ot[:], acc[a][:],
                                            bt[:, q:q + 1], 0.0,
                                            mybir.AluOpType.add,
                                            mybir.AluOpType.max)
                    nc.sync.dma_start(o_d[q], ot[:])
    if bounds:
        n_mm = sum(len(g) for g in sched)
        _inject_round_waits(nc, bounds, n_mm)
    _legalize_waits(nc)
    if not os.environ.get("BASS_KERNEL_NO_THIN"):
        _thin_pe_incs(nc)
    return nc


def _prep_inputs(x, w_blocks, bias, row_idx, col_idx, slot_of, dummy_slots, S):
    nnz = len(row_idx)
    # x^T images per core: [128, 32, BC] -> block i at partitions 32*(i%4),
    # free tile i//4.  x[b, 32*(4t+r)+p] -> xt[32r+p, t, b]
    xb = x.astype(IN_NP).reshape(BATCH, 32, 4, 32)        # b, t, r, p
    xt_all = np.ascontiguousarray(xb.transpose(2, 3, 1, 0))  # r, p, t, b
    xt_all = xt_all.reshape(128, 32, BATCH)
    xts = [np.ascontiguousarray(xt_all[:, :, c * BC:(c + 1) * BC]
                                ).reshape(128, 32 * BC) for c in range(NCORES)]
    # w image [128, S*32]: block n at partition strip 32*(row%4), slot
    wim = np.zeros((128, S * 32), dtype=IN_NP)
    wb = w_blocks.astype(IN_NP)
    for n in range(nnz):
        r = int(row_idx[n]) % 4
        s = slot_of[n]
        wim[32 * r:32 * r + 32, 32 * s:32 * s + 32] = wb[n]
    # dummy slots already zero
    bim = np.ascontiguousarray(
        bias.astype(np.float32).reshape(32, 4, 32).transpose(1, 2, 0)
    ).reshape(128, 32)
    return xts, wim, bim


def kernel(x, w_blocks, bias, row_idx, col_idx):
    repeat = int(os.environ.get("BASS_KERNEL_REPEAT", "1"))
    key = (row_idx.tobytes(), col_idx.tobytes(), repeat)
    if key not in _CACHE:
        sched, S, slot_of, dummy_slots, bounds = _build_schedule(row_idx,
                                                                 col_idx)
        nc = _build_program(sched, S, repeat=repeat, bounds=bounds)
        _CACHE[key] = (nc, S, (slot_of, dummy_slots))
    nc, S, aux = _CACHE[key]

    slot_of, dummy_slots = aux
    xts, wim, bim = _prep_inputs(x, w_blocks, bias, row_idx, col_idx,
                                 slot_of, dummy_slots, S)
    in_maps = [{"xt": xts[c], "wim": wim, "bias": bim} for c in range(NCORES)]
    trace = bool(os.environ.get("BASS_KERNEL_TRACE"))
    res = run_bass_kernel_spmd(nc, in_maps, list(range(NCORES)), trace=trace)
    global LAST_RESULTS
    LAST_RESULTS = res

    out = np.empty((BATCH, NB * BS), dtype=np.float32)
    for c in range(NCORES):
        outT = res.results[c]["outT"].reshape(NB * BS, BC)
        out[c * BC:(c + 1) * BC, :] = outT.T.astype(np.float32)
    return out
